# revision 49
# baseline (speedup 1.0000x reference)
"""Trainium2 Bass kernel for a single transformer decoder layer.

Reference semantics (B=64, T=200, E=512, H=8, D=64):
  x += SelfAttn(LN1(x))   (q,k row-masked by pred_mask, causal)
  x += CrossAttn(LN2(x))  (k from raw memory row-masked by src_mask,
                           v from LN2(x) (!), causal)
  x += FFN(LN3(x))        (512 -> 2048 -> relu -> 512)

Sharding: data-parallel over batch, 8 elems per NeuronCore, no collectives.

Design (v4, fp8 + stage-batched):
  - residual stream x NATURAL [tc<=128, 512] fp32; LN via bn_stats+Rsqrt
  - all 4 pairs are emitted stage-by-stage (SA for all pairs, then CA,
    then FFN) so each engine's FIFO interleaves independent work and the
    PE never cools (HAM stays at full clock)
  - h cast bf16, PE-transposed (4 transposes into one PSUM bank, one
    drain), drained to fp8e4 tiles hT [128, 4(c), 400]
  - all six GEMM families (Q,K,V,O,W1,W2) run fp8 DoubleRow (K=256 per
    instruction): weights pre-scaled x64 host-side (fp8e4 normal range)
  - Q/K drains split per 64-row head half into [64, 2, 400] bf16 tiles
    (base partition 0); SA pred_mask rides the drain as a
    scalar_tensor_tensor multiply; CA drains on the ACT engine
  - softmax denominators: ones-stationary matmuls into a [4(oc), 2(hl),
    200] PSUM tile (8 matmuls), reciprocal_approx_fast, bf16 cast, then
    8 small PE broadcast matmuls -> dbc [128, 200] per oc
  - weight/mem/out DMAs issued from the gpsimd queue (idle), x/pm/sm on
    the sync queue; x/mem/sm host-padded to 256 rows for 1-DMA loads
  - causal mask via gpsimd.affine_select(fill=0) after exp (scores O(1))
"""

import numpy as np
import ml_dtypes
from contextlib import ExitStack

import concourse.bass as bass
import concourse.bacc as bacc
import concourse.tile as tile
from concourse import mybir
from concourse.bass_utils import run_bass_kernel_spmd

B, T, E, H, Dh, F = 64, 200, 512, 8, 64, 2048
NCORES = 8
SCALE = float(E) ** -0.5
WS = 64.0  # fp8 weight pre-scale
F32 = mybir.dt.float32
BF16 = mybir.dt.bfloat16
F8 = mybir.dt.float8e4
AL = mybir.AluOpType
AF = mybir.ActivationFunctionType
DR = mybir.MatmulPerfMode.DoubleRow
TCH = [(0, 128), (128, 72)]  # token chunks (t0, tc)
NPBF16 = ml_dtypes.bfloat16
NPF8 = ml_dtypes.float8_e4m3fn
T2 = 2 * T

_programs = {}


def _ln_pair(nc, pools, x_pair, eps):
    """LN over 2 elems x 2 chunks, ACT functions grouped to limit
    activation-table swaps. Returns 2x2 bf16 h chunks."""
    ch = []
    for el in range(2):
        xs = x_pair[el]
        for ci, (t0, tc) in enumerate(TCH):
            x_c = xs[0:tc, ci, :] if not isinstance(xs, list) else xs[ci][:, :]
            ch.append((x_c, tc))
    mvs = []
    for x_c, tc in ch:
        st6 = pools["small"].tile([tc, 6], F32, name="st6")
        nc.vector.bn_stats(st6[:, :], x_c)
        mv = pools["small"].tile([tc, 2], F32, name="mv")
        nc.vector.bn_aggr(mv[:, :], st6[:, :])
        mvs.append(mv)
    stds = []
    for (x_c, tc), mv in zip(ch, mvs):
        std = pools["small"].tile([tc, 1], F32, name="std")
        nc.scalar.activation(std[:, :], mv[:, 1:2], AF.Sqrt,
                             bias=eps[0:tc, 0:1])
        stds.append(std)
    abs_ = []
    for (x_c, tc), mv, std in zip(ch, mvs, stds):
        rstd = pools["small"].tile([tc, 1], F32, name="rstd")
        nc.vector.reciprocal(rstd[:, :], std[:, :])
        nb = pools["small"].tile([tc, 1], F32, name="nb")
        nc.vector.tensor_scalar(nb[:, :], mv[:, 0:1], rstd[:, 0:1], -1.0,
                                op0=AL.mult, op1=AL.mult)
        abs_.append((rstd, nb))
    out = []
    for el in range(2):
        hs = []
        for ci in range(2):
            i = el * 2 + ci
            (x_c, tc), (rstd, nb) = ch[i], abs_[i]
            h_c = pools["h"].tile([tc, E], BF16, name="h_c", tag="h_c",
                                  bufs=6)
            nc.scalar.activation(h_c[:, :], x_c, AF.Identity,
                                 scale=rstd[:, 0:1], bias=nb[:, 0:1])
            hs.append(h_c)
        out.append(hs)
    return out


def _transpose_f8(nc, pools, h_cs_pair, ident):
    """pair of 2 elems x 2 chunks of [tc,512] bf16 natural ->
    hT [128, 4(c), 400] fp8 tile via PE transposes (4 per PSUM bank)."""
    hT = pools["tT"].tile([128, 4, T2], F8, name="hT", tag="tT", bufs=9)
    for el in range(2):
        for ci, (t0, tc) in enumerate(TCH):
            ps = pools["ps"].tile([128, 4, tc], BF16, name="t_ps", tag="ps")
            for ec in range(4):
                nc.tensor.transpose(
                    ps[:, ec, :], h_cs_pair[el][ci][0:tc, ec * 128:(ec + 1) * 128],
                    ident[0:tc, 0:tc])
            nc.vector.tensor_copy(hT[:, :, el * T + t0:el * T + t0 + tc],
                                  ps[:, :, :])
    return hT


def _project_qk(nc, pools, w_sb, hT, name, mask_bc=None):
    """fp8 DoubleRow projection -> per-oc [64, 2(head-half), 400] bf16
    tiles (base partition 0). mask_bc: [64, 400] bf16 multiplied in."""
    out = []
    for oc in range(4):
        ps = pools["ps"].tile([128, T2], F32, name=f"{name}_ps", tag="ps")
        nc.tensor.matmul(ps[:, :], w_sb[:, 0:2, oc * 128:(oc + 1) * 128],
                         hT[:, 0:2, :], start=True, stop=False, perf_mode=DR)
        nc.tensor.matmul(ps[:, :], w_sb[:, 2:4, oc * 128:(oc + 1) * 128],
                         hT[:, 2:4, :], start=False, stop=True, perf_mode=DR)
        sb = pools["qk"].tile([64, 2, T2], F8, name=f"{name}_sb", tag="qk",
                              bufs=24)
        for hl in range(2):
            hp = hl * 64
            if mask_bc is not None:
                nc.vector.scalar_tensor_tensor(
                    sb[:, hl, :], ps[hp:hp + 64, :], 1.0 / WS, mask_bc[0:64, :],
                    op0=AL.mult, op1=AL.mult)
            else:
                nc.scalar.activation(sb[:, hl, :], ps[hp:hp + 64, :],
                                     AF.Identity, scale=1.0 / WS)
        out.append(sb)
    return out


def _project_v(nc, pools, wv_sb, hT, off, name):
    """fp8 DoubleRow -> v_dr [128, 2(s-sub), 512] fp8 (WS-scaled), sub 1
    rows 72:128 zeroed (token pad)."""
    v_dr = pools["v"].tile([128, 2, E], F8, name=f"{name}_dr", tag="v",
                           bufs=6)
    nc.gpsimd.memset(v_dr[64:128, 1, :], 0.0)
    for ci, (t0, tc) in enumerate(TCH):
        ps = pools["ps"].tile([tc, E], F32, name=f"{name}_ps", tag="ps")
        nc.tensor.matmul(ps[:, :], hT[:, 0:2, off + t0:off + t0 + tc],
                         wv_sb[:, 0:2, :], start=True, stop=False, perf_mode=DR)
        nc.tensor.matmul(ps[:, :], hT[:, 2:4, off + t0:off + t0 + tc],
                         wv_sb[:, 2:4, :], start=False, stop=True, perf_mode=DR)
        nc.scalar.copy(v_dr[0:tc, ci, :], ps[:, :])
    return v_dr


def _attn_stage(nc, pools, P, QT, KT, VV, onesdr, wo_sb, XCS):
    """One attention stage for all pairs/elems, phase-major, fp8 e/v with
    DoubleRow AV over the two key-position subtiles."""
    keys = [(pr, el) for pr in P for el in range(2)]
    steps = [(k, oc) for oc in range(4) for k in keys]
    ES = {}
    OT = {k: [None] * 4 for k in keys}
    LAG = 4

    def emit_scores(k, oc):
        pr, el = k
        off = el * T
        qt, kt = QT[pr], KT[pr]
        st0 = pools["ps"].tile([128, 2, 200], F32, name="st0", tag="ps")
        st1 = pools["ps"].tile([72, 2, 72], F32, name="st1", tag="ps")
        for hl in range(2):
            qh = qt[oc][0:64, hl, off:off + 200]
            kh = kt[oc][0:64, hl, off:off + 200]
            nc.tensor.matmul(st0[:, hl, :], kh[:, 0:128], qh)
            nc.tensor.matmul(st1[:, hl, :], kh[:, 128:200], qh[:, 128:200])
        e_dr = pools["e0"].tile([128, 2, 2, 208], F8, name="e_dr", bufs=3)
        nc.scalar.activation(e_dr[:, 0, :, 0:200], st0[:, :, :], AF.Exp,
                             scale=SCALE)
        nc.scalar.activation(e_dr[0:72, 1, :, 128:200], st1[:, :, :],
                             AF.Exp, scale=SCALE)
        e_sel = pools["e0"].tile([128, 2, 2, 208], F8, name="e_sel", bufs=12)
        nc.gpsimd.memset(e_sel[:, 1, :, :], 0.0)
        nc.gpsimd.affine_select(
            e_sel[:, 0, :, 0:200], e_dr[:, 0, :, 0:200],
            pattern=[[0, 2], [1, 200]], compare_op=AL.is_ge, fill=0.0,
            base=0, channel_multiplier=-1)
        nc.gpsimd.affine_select(
            e_sel[0:72, 1, :, 128:200], e_dr[0:72, 1, :, 128:200],
            pattern=[[0, 2], [1, 72]], compare_op=AL.is_ge, fill=0.0,
            base=0, channel_multiplier=-1)
        ES[(k, oc)] = e_sel

    def emit_av(k, oc):
        v_dr = VV[k]
        es = ES[(k, oc)]
        av = pools["ps"].tile([64, 2, 200], F32, name="av", tag="ps")
        drep = pools["ps"].tile([64, 2, 200], F32, name="drep", tag="ps")
        for hl in range(2):
            h = 2 * oc + hl
            nc.tensor.matmul(av[:, hl, :], v_dr[:, :, h * 64:(h + 1) * 64],
                             es[:, :, hl, 0:200],
                             skip_group_check=True, perf_mode=DR)
        nc.tensor.matmul(drep[:, :, :], onesdr[:, :, :], es[:, :, :, 0:200],
                         skip_group_check=True, perf_mode=DR)
        dinv = pools["dbc"].tile([64, 2, 200], F32, name="dinv", bufs=6)
        nc.vector.reciprocal_approx_fast(dinv[:, :, :], drep[:, :, :])
        ot = pools["ot"].tile([64, 2, 208], F8, name="ot", bufs=16)
        nc.vector.tensor_mul(ot[:, :, 0:200], av[:, :, :], dinv[:, :, :])
        OT[k][oc] = ot

    for i, (k, oc) in enumerate(steps):
        emit_scores(k, oc)
        if i >= LAG:
            emit_av(*steps[i - LAG])
    for i in range(len(steps) - LAG, len(steps)):
        emit_av(*steps[i])

    XN = {}
    for k in keys:
        new_x = []
        for ci, (t0, tc) in enumerate(TCH):
            ps = pools["ps"].tile([tc, E], F32, name="proj_ps", tag="ps")
            for oc in range(4):
                nc.tensor.matmul(ps[:, :], OT[k][oc][0:64, :, t0:t0 + tc],
                                 wo_sb[0:64, 2 * oc:2 * oc + 2, :],
                                 start=(oc == 0), stop=(oc == 3), perf_mode=DR)
            xn = pools["res"].tile([tc, E], F32, name="xn", tag="res")
            nc.vector.scalar_tensor_tensor(xn[:, :], ps[:, :],
                                           1.0 / (WS * WS), XCS[k][ci],
                                           op0=AL.mult, op1=AL.add)
            new_x.append(xn)
        XN[k] = new_x
    return XN


def _build(bpc, stages=3):
    nc = bacc.Bacc("TRN2", target_bir_lowering=False, debug=False,
                   enable_asserts=False, num_devices=NCORES)

    def din(name, shape, dt):
        return nc.dram_tensor(name, list(shape), dt, kind="ExternalInput")

    x_d = din("x", (bpc, 2, 128, E), F32)       # host-padded 200 -> 256 rows
    mem_d = din("mem", (bpc, 2, 128, E), BF16)  # host-padded
    pm_d = din("pm", (bpc // 2, 1, T2), BF16)   # per-pair row
    sm_d = din("sm", (bpc // 2, 1, T2), BF16)   # per-pair row
    wq_sa_d = din("wq_sa", (E, E), F8)
    wk_sa_d = din("wk_sa", (E, E), F8)
    wv_sa_d = din("wv_sa", (E, E), F8)
    wo_sa_d = din("wo_sa", (E, E), F8)
    wq_ca_d = din("wq_ca", (E, E), F8)
    wk_ca_d = din("wk_ca", (E, E), F8)
    wv_ca_d = din("wv_ca", (E, E), F8)
    wo_ca_d = din("wo_ca", (E, E), F8)
    w1_d = din("w1", (E, F), F8)
    w2_d = din("w2", (F, E), F8)
    b1_d = din("b1", (1, F), F32)  # WS*(f_b1 + ln3_b @ f_w1), column bias
    out_d = nc.dram_tensor("out", [bpc, T, E], F32, kind="ExternalOutput")

    identb_d = nc.inline_tensor(np.eye(128, dtype=NPBF16), name="identbc")
    onesdr_np = np.ones((128, 2, 64), dtype=NPF8)
    onesdr_np[72:128, 1, :] = 0  # token-pad rows of key-subtile 1
    onesdr_d = nc.inline_tensor(onesdr_np, name="onesdrc")

    npairs = bpc // 2

    with tile.TileContext(nc) as tcx, ExitStack() as ctx:
        pools = {}

        def pool(name, bufs, space="SBUF"):
            pools[name] = ctx.enter_context(
                tcx.tile_pool(name=name, bufs=bufs, space=space))
            return pools[name]

        wpool = pool("w", 1)
        pool("small", 10)
        pool("h", 8)
        pool("m", 5)
        pool("x2", 6)
        pool("tT", 9)
        pool("qk", 18)
        pool("v", 8)
        pool("e0", 6)
        pool("e1", 6)
        pool("ot", 10)
        pool("dbc", 5)
        pool("res", 15)
        pool("rT", 17)
        pool("mrow", 4)
        pool("ps", 8, space="PSUM")

        eps = wpool.tile([128, 1], F32, tag="eps", bufs=1, name="eps")
        nc.gpsimd.memset(eps[:, :], 1e-5)

        def wtile(name, src, shape, rearr=None, dt=F8, eng=None):
            t = wpool.tile(shape, dt, tag=name, bufs=1, name=name)
            ap = src[:] if rearr is None else src[:].rearrange(rearr, p=shape[0])
            (eng or nc.gpsimd).dma_start(t[...], ap)
            return t

        # consts on sync (tiny, needed early); weights on gpsimd queue
        identb = wtile("identb", identb_d, [128, 128], dt=BF16)
        sel4 = wtile("sel4", sel4_d, [128, 4, 2, 16], dt=F8)
        ones4 = wtile("ones4", ones4_d, [4, 4, 64], dt=BF16)
        wq_sa = wtile("wq_sa", wq_sa_d, [128, 4, E], "(c p) n -> p c n")
        wk_sa = wtile("wk_sa", wk_sa_d, [128, 4, E], "(c p) n -> p c n")
        wv_sa = wtile("wv_sa", wv_sa_d, [128, 4, E], "(c p) n -> p c n")
        wo_sa = wtile("wo_sa", wo_sa_d, [64, 8, E], "(c p) n -> p c n")
        wq_ca = wtile("wq_ca", wq_ca_d, [128, 4, E], "(c p) n -> p c n")
        wk_ca = wtile("wk_ca", wk_ca_d, [128, 4, E], "(c p) n -> p c n")
        wv_ca = wtile("wv_ca", wv_ca_d, [128, 4, E], "(c p) n -> p c n")
        wo_ca = wtile("wo_ca", wo_ca_d, [64, 8, E], "(c p) n -> p c n")
        w1 = wtile("w1", w1_d, [128, 4, F], "(c p) n -> p c n")
        w2 = wtile("w2", w2_d, [128, 16, E], "(c p) n -> p c n")
        b1c = wtile("b1c", b1_d, [128, 16], dt=F32, rearr="o (c p) -> p (o c)")

        GS = {}  # per-group state

        def emit_loads(g):
            st = GS[g] = {"X": {}, "PMBC": {}, "SMBC": {}, "M2": {},
                          "HT": {}, "QT": {}, "KT": {}, "MT": {}}
            for pr in (2 * g, 2 * g + 1):
                els = (2 * pr, 2 * pr + 1)
                x_el = []
                for el, e in enumerate(els):
                    x2 = pools["x2"].tile([128, 2, E], F32, name="x_in",
                                          tag="x2")
                    nc.sync.dma_start(
                        x2[...], x_d[e].rearrange("c p n -> p c n", p=128))
                    x_el.append(x2)
                st["X"][pr] = x_el
                pmrow2 = pools["mrow"].tile([1, T2], BF16, name="pmrow2", bufs=2)
                nc.sync.dma_start(pmrow2[0:1, :], pm_d[pr, :, :])
                pm_bc = pools["mrow"].tile([64, T2], BF16, name="pm_bc")
                nc.gpsimd.partition_broadcast(pm_bc[0:64, :], pmrow2[0:1, :])
                st["PMBC"][pr] = pm_bc
                smrow2 = pools["mrow"].tile([1, T2], BF16, name="smrow2", bufs=2)
                nc.sync.dma_start(smrow2[0:1, :], sm_d[pr, :, :])
                sm_bc = pools["mrow"].tile([64, T2], BF16, name="sm_bc")
                nc.gpsimd.partition_broadcast(sm_bc[0:64, :], smrow2[0:1, :])
                st["SMBC"][pr] = sm_bc

        def emit_sa_ln(g):
            st = GS[g]
            for pr in (2 * g, 2 * g + 1):
                h_pair = _ln_pair(nc, pools, st["X"][pr], eps)
                st["HT"][pr] = _transpose_f8(nc, pools, h_pair, identb)

        def emit_sa_qk(g):
            st = GS[g]
            for pr in (2 * g, 2 * g + 1):
                st["QT"][pr] = _project_qk(nc, pools, wq_sa, st["HT"][pr],
                                           "q_sa", mask_bc=st["PMBC"][pr])
                st["KT"][pr] = _project_qk(nc, pools, wk_sa, st["HT"][pr],
                                           "k_sa", mask_bc=st["PMBC"][pr])

        def emit_sa_attn(g):
            st = GS[g]
            P = (2 * g, 2 * g + 1)
            # mem loads for CA ride along here (gpsimd queue, long lead)
            for pr in P:
                m_el = []
                for el, e in enumerate((2 * pr, 2 * pr + 1)):
                    m2 = pools["m"].tile([128, 2, E], BF16, name="m_nat",
                                         tag="m_nat")
                    nc.gpsimd.dma_start(
                        m2[...], mem_d[e].rearrange("c p n -> p c n", p=128))
                    m_el.append(m2)
                st["M2"][pr] = m_el
            VV = {}
            XCS = {}
            for pr in P:
                for el in range(2):
                    VV[(pr, el)] = _project_v(nc, pools, wv_sa, st["HT"][pr],
                                              el * T, "v_sa")
                    xs = st["X"][pr][el]
                    XCS[(pr, el)] = [xs[0:tc, ci, :]
                                     for ci, (t0, tc) in enumerate(TCH)]
            XN = _attn_stage(nc, pools, P, st["QT"], st["KT"], VV, onesdr,
                             wo_sa, XCS)
            for pr in P:
                st["X"][pr] = [XN[(pr, 0)], XN[(pr, 1)]]

        def emit_ca_prep(g):
            st = GS[g]
            P = (2 * g, 2 * g + 1)
            for pr in P:
                h_pair = _ln_pair(nc, pools, st["X"][pr], eps)
                st["HT"][pr] = _transpose_f8(nc, pools, h_pair, identb)
            for pr in P:
                m_views = [[st["M2"][pr][el][0:128, ci, :] for ci in range(2)]
                           for el in range(2)]
                st["MT"][pr] = _transpose_f8(nc, pools, m_views, identb)
            for pr in P:
                st["QT"][pr] = _project_qk(nc, pools, wq_ca, st["HT"][pr],
                                           "q_ca")
                st["KT"][pr] = _project_qk(nc, pools, wk_ca, st["MT"][pr],
                                           "k_ca", mask_bc=st["SMBC"][pr])

        def emit_ca_attn(g):
            st = GS[g]
            P = (2 * g, 2 * g + 1)
            VV = {}
            XCS = {}
            for pr in P:
                for el in range(2):
                    VV[(pr, el)] = _project_v(nc, pools, wv_ca, st["HT"][pr],
                                              el * T, "v_ca")
                    XCS[(pr, el)] = st["X"][pr][el]
            XN = _attn_stage(nc, pools, P, st["QT"], st["KT"], VV, onesdr,
                             wo_ca, XCS)
            for pr in P:
                st["X"][pr] = [XN[(pr, 0)], XN[(pr, 1)]]

        def emit_ffn(g):
            st = GS[g]
            P = (2 * g, 2 * g + 1)
            for pr in P:
                h_pair = _ln_pair(nc, pools, st["X"][pr], eps)
                st["HT"][pr] = _transpose_f8(nc, pools, h_pair, identb)
            for pr in P:
                rT = []
                for fp in range(8):
                    r = pools["rT"].tile([128, 2, T2], F8, name="r")
                    for sub in range(2):
                        fc = 2 * fp + sub
                        zps = pools["ps"].tile([128, T2], F32, name="z_ps",
                                               tag="ps")
                        nc.tensor.matmul(zps[:, :],
                                         w1[:, 0:2, fc * 128:(fc + 1) * 128],
                                         st["HT"][pr][:, 0:2, :], start=True,
                                         stop=False, perf_mode=DR)
                        nc.tensor.matmul(zps[:, :],
                                         w1[:, 2:4, fc * 128:(fc + 1) * 128],
                                         st["HT"][pr][:, 2:4, :], start=False,
                                         stop=True, perf_mode=DR)
                        nc.scalar.activation(r[:, sub, :], zps[:, :],
                                             AF.Relu, bias=b1c[:, fc:fc + 1])
                    rT.append(r)
                for el in range(2):
                    e = 2 * pr + el
                    off = el * T
                    for ci, (t0, tc) in enumerate(TCH):
                        yps = pools["ps"].tile([tc, E], F32, name="y_ps",
                                               tag="ps")
                        for fp in range(8):
                            nc.tensor.matmul(
                                yps[:, :],
                                rT[fp][:, :, off + t0:off + t0 + tc],
                                w2[:, 2 * fp:2 * fp + 2, :],
                                start=(fp == 0), stop=(fp == 7), perf_mode=DR)
                        yout = pools["res"].tile([tc, E], F32, name="yout",
                                                 tag="res")
                        nc.vector.scalar_tensor_tensor(
                            yout[:, :], yps[:, :], 1.0 / (WS * WS),
                            st["X"][pr][el][ci][:, :], op0=AL.mult, op1=AL.add)
                        nc.gpsimd.dma_start(out_d[e, t0:t0 + tc, :],
                                            yout[:, :])

        # software-pipelined schedule across the two groups
        emit_loads(0)
        emit_sa_ln(0)
        emit_sa_qk(0)
        emit_sa_attn(0)
        emit_loads(1)
        emit_sa_ln(1)
        emit_ca_prep(0)
        emit_ca_attn(0)
        emit_sa_qk(1)
        emit_ffn(0)
        emit_sa_attn(1)
        emit_ca_prep(1)
        emit_ca_attn(1)
        emit_ffn(1)

    nc.compile()
    return nc


def _host_prep(inputs, bpc, core):
    """Build the in_map for one core."""
    s = slice(core * bpc, (core + 1) * bpc)

    def rearr(w, g=None):  # (H, E, D) -> [E, H*D], optionally row-scaled
        m = np.transpose(np.asarray(w, np.float32), (1, 0, 2)).reshape(E, E)
        if g is not None:
            m = m * np.asarray(g, np.float32)[:, None]
        return m

    def f8(a):  # scale x64, clip to TRN fp8e4 range, cast
        return np.clip(np.asarray(a, np.float32) * WS,
                       -240.0, 240.0).astype(NPF8)

    def pad256(a, dt):  # [bpc, T, c] -> [bpc, 2, 128, c]
        out = np.zeros((bpc, 256, a.shape[2]), dtype=dt)
        out[:, :T, :] = a
        return np.ascontiguousarray(out.reshape(bpc, 2, 128, a.shape[2]))

    g1 = np.asarray(inputs["ln1_g"], np.float32)
    g2 = np.asarray(inputs["ln2_g"], np.float32)
    g3 = np.asarray(inputs["ln3_g"], np.float32)
    b3n = np.asarray(inputs["ln3_b"], np.float32)
    w1f = np.asarray(inputs["f_w1"], np.float32)
    b1f = (np.asarray(inputs["f_b1"], np.float32) + b3n @ w1f) * WS

    return {
        "x": pad256(np.asarray(inputs["idx"], np.float32)[s], np.float32),
        "mem": pad256(np.asarray(inputs["memory"], np.float32)[s], NPBF16),
        "pm": np.ascontiguousarray(
            (np.asarray(inputs["pred_mask"])[s] != 0).astype(NPBF16)
            .reshape(bpc // 2, 1, T2)),
        "sm": np.ascontiguousarray(
            (np.asarray(inputs["src_mask"])[s] != 0).astype(NPBF16)
            .reshape(bpc // 2, 1, T2)),
        "wq_sa": f8(rearr(inputs["sa_wq"], g1)),
        "wk_sa": f8(rearr(inputs["sa_wk"], g1)),
        "wv_sa": f8(rearr(inputs["sa_wv"], g1)),
        "wo_sa": f8(inputs["sa_wo"]),
        "wq_ca": f8(rearr(inputs["ca_wq"], g2)),
        "wk_ca": f8(rearr(inputs["ca_wk"])),
        "wv_ca": f8(rearr(inputs["ca_wv"], g2)),
        "wo_ca": f8(inputs["ca_wo"]),
        "w1": f8(w1f * g3[:, None]),
        "w2": f8(inputs["f_w2"]),
        "b1": np.ascontiguousarray(b1f.reshape(1, F)),
    }


def get_program(bpc):
    if bpc not in _programs:
        _programs[bpc] = _build(bpc)
    return _programs[bpc]


def kernel(**inputs) -> np.ndarray:
    bpc = B // NCORES
    nc = get_program(bpc)
    in_maps = [_host_prep(inputs, bpc, c) for c in range(NCORES)]
    res = run_bass_kernel_spmd(nc, in_maps, core_ids=list(range(NCORES)))
    out = np.concatenate([res.results[c]["out"] for c in range(NCORES)], axis=0)
    return out.astype(np.float32)


# revision 50
# speedup vs baseline: 1.0104x; 1.0104x over previous
"""Trainium2 Bass kernel for a single transformer decoder layer.

Reference semantics (B=64, T=200, E=512, H=8, D=64):
  x += SelfAttn(LN1(x))   (q,k row-masked by pred_mask, causal)
  x += CrossAttn(LN2(x))  (k from raw memory row-masked by src_mask,
                           v from LN2(x) (!), causal)
  x += FFN(LN3(x))        (512 -> 2048 -> relu -> 512)

Sharding: data-parallel over batch, 8 elems per NeuronCore, no collectives.

Design (v4, fp8 + stage-batched):
  - residual stream x NATURAL [tc<=128, 512] fp32; LN via bn_stats+Rsqrt
  - all 4 pairs are emitted stage-by-stage (SA for all pairs, then CA,
    then FFN) so each engine's FIFO interleaves independent work and the
    PE never cools (HAM stays at full clock)
  - h cast bf16, PE-transposed (4 transposes into one PSUM bank, one
    drain), drained to fp8e4 tiles hT [128, 4(c), 400]
  - all six GEMM families (Q,K,V,O,W1,W2) run fp8 DoubleRow (K=256 per
    instruction): weights pre-scaled x64 host-side (fp8e4 normal range)
  - Q/K drains split per 64-row head half into [64, 2, 400] bf16 tiles
    (base partition 0); SA pred_mask rides the drain as a
    scalar_tensor_tensor multiply; CA drains on the ACT engine
  - softmax denominators: ones-stationary matmuls into a [4(oc), 2(hl),
    200] PSUM tile (8 matmuls), reciprocal_approx_fast, bf16 cast, then
    8 small PE broadcast matmuls -> dbc [128, 200] per oc
  - weight/mem/out DMAs issued from the gpsimd queue (idle), x/pm/sm on
    the sync queue; x/mem/sm host-padded to 256 rows for 1-DMA loads
  - causal mask via gpsimd.affine_select(fill=0) after exp (scores O(1))
"""

import numpy as np
import ml_dtypes
from contextlib import ExitStack

import concourse.bass as bass
import concourse.bacc as bacc
import concourse.tile as tile
from concourse import mybir
from concourse.bass_utils import run_bass_kernel_spmd

B, T, E, H, Dh, F = 64, 200, 512, 8, 64, 2048
NCORES = 8
SCALE = float(E) ** -0.5
WS = 64.0  # fp8 weight pre-scale
F32 = mybir.dt.float32
BF16 = mybir.dt.bfloat16
F8 = mybir.dt.float8e4
AL = mybir.AluOpType
AF = mybir.ActivationFunctionType
DR = mybir.MatmulPerfMode.DoubleRow
TCH = [(0, 128), (128, 72)]  # token chunks (t0, tc)
NPBF16 = ml_dtypes.bfloat16
NPF8 = ml_dtypes.float8_e4m3fn
T2 = 2 * T

_programs = {}


def _ln_pair(nc, pools, x_pair, eps):
    """LN over 2 elems x 2 chunks, ACT functions grouped to limit
    activation-table swaps. Returns 2x2 bf16 h chunks."""
    ch = []
    for el in range(2):
        xs = x_pair[el]
        for ci, (t0, tc) in enumerate(TCH):
            x_c = xs[0:tc, ci, :] if not isinstance(xs, list) else xs[ci][:, :]
            ch.append((x_c, tc))
    mvs = []
    for x_c, tc in ch:
        st6 = pools["small"].tile([tc, 6], F32, name="st6")
        nc.vector.bn_stats(st6[:, :], x_c)
        mv = pools["small"].tile([tc, 2], F32, name="mv")
        nc.vector.bn_aggr(mv[:, :], st6[:, :])
        mvs.append(mv)
    stds = []
    for (x_c, tc), mv in zip(ch, mvs):
        std = pools["small"].tile([tc, 1], F32, name="std")
        nc.scalar.activation(std[:, :], mv[:, 1:2], AF.Sqrt,
                             bias=eps[0:tc, 0:1])
        stds.append(std)
    abs_ = []
    for (x_c, tc), mv, std in zip(ch, mvs, stds):
        rstd = pools["small"].tile([tc, 1], F32, name="rstd")
        nc.vector.reciprocal(rstd[:, :], std[:, :])
        nb = pools["small"].tile([tc, 1], F32, name="nb")
        nc.vector.tensor_scalar(nb[:, :], mv[:, 0:1], rstd[:, 0:1], -1.0,
                                op0=AL.mult, op1=AL.mult)
        abs_.append((rstd, nb))
    out = []
    for el in range(2):
        hs = []
        for ci in range(2):
            i = el * 2 + ci
            (x_c, tc), (rstd, nb) = ch[i], abs_[i]
            h_c = pools["h"].tile([tc, E], BF16, name="h_c", tag="h_c",
                                  bufs=6)
            nc.scalar.activation(h_c[:, :], x_c, AF.Identity,
                                 scale=rstd[:, 0:1], bias=nb[:, 0:1])
            hs.append(h_c)
        out.append(hs)
    return out


def _transpose_f8(nc, pools, h_cs_pair, ident):
    """pair of 2 elems x 2 chunks of [tc,512] bf16 natural ->
    hT [128, 4(c), 400] fp8 tile via PE transposes (4 per PSUM bank)."""
    hT = pools["tT"].tile([128, 4, T2], F8, name="hT", tag="tT", bufs=9)
    for el in range(2):
        for ci, (t0, tc) in enumerate(TCH):
            ps = pools["ps"].tile([128, 4, tc], BF16, name="t_ps", tag="ps")
            for ec in range(4):
                nc.tensor.transpose(
                    ps[:, ec, :], h_cs_pair[el][ci][0:tc, ec * 128:(ec + 1) * 128],
                    ident[0:tc, 0:tc])
            nc.vector.tensor_copy(hT[:, :, el * T + t0:el * T + t0 + tc],
                                  ps[:, :, :])
    return hT


def _project_qk(nc, pools, w_sb, hT, name, mask_bc=None):
    """fp8 DoubleRow projection -> per-oc [64, 2(head-half), 400] bf16
    tiles (base partition 0). mask_bc: [64, 400] bf16 multiplied in."""
    out = []
    for oc in range(4):
        ps = pools["ps"].tile([128, T2], F32, name=f"{name}_ps", tag="ps")
        nc.tensor.matmul(ps[:, :], w_sb[:, 0:2, oc * 128:(oc + 1) * 128],
                         hT[:, 0:2, :], start=True, stop=False, perf_mode=DR)
        nc.tensor.matmul(ps[:, :], w_sb[:, 2:4, oc * 128:(oc + 1) * 128],
                         hT[:, 2:4, :], start=False, stop=True, perf_mode=DR)
        sb = pools["qk"].tile([64, 2, T2], F8, name=f"{name}_sb", tag="qk",
                              bufs=24)
        for hl in range(2):
            hp = hl * 64
            if mask_bc is not None:
                nc.vector.scalar_tensor_tensor(
                    sb[:, hl, :], ps[hp:hp + 64, :], 1.0 / WS, mask_bc[0:64, :],
                    op0=AL.mult, op1=AL.mult)
            else:
                nc.scalar.activation(sb[:, hl, :], ps[hp:hp + 64, :],
                                     AF.Identity, scale=1.0 / WS)
        out.append(sb)
    return out


def _project_v(nc, pools, wv_sb, hT, off, name):
    """fp8 DoubleRow -> v_dr [128, 2(s-sub), 512] fp8 (WS-scaled), sub 1
    rows 72:128 zeroed (token pad)."""
    v_dr = pools["v"].tile([128, 2, E], F8, name=f"{name}_dr", tag="v",
                           bufs=6)
    nc.gpsimd.memset(v_dr[64:128, 1, :], 0.0)
    for ci, (t0, tc) in enumerate(TCH):
        ps = pools["ps"].tile([tc, E], F32, name=f"{name}_ps", tag="ps")
        nc.tensor.matmul(ps[:, :], hT[:, 0:2, off + t0:off + t0 + tc],
                         wv_sb[:, 0:2, :], start=True, stop=False, perf_mode=DR)
        nc.tensor.matmul(ps[:, :], hT[:, 2:4, off + t0:off + t0 + tc],
                         wv_sb[:, 2:4, :], start=False, stop=True, perf_mode=DR)
        nc.scalar.copy(v_dr[0:tc, ci, :], ps[:, :])
    return v_dr


def _attn_stage(nc, pools, P, QT, KT, VV, onesdr, wo_sb, XCS):
    """One attention stage for all pairs/elems, phase-major, fp8 e/v with
    DoubleRow AV over the two key-position subtiles."""
    keys = [(pr, el) for pr in P for el in range(2)]
    steps = [(k, oc) for oc in range(4) for k in keys]
    ES = {}
    OT = {k: [None] * 4 for k in keys}
    LAG = 4

    def emit_scores(k, oc):
        pr, el = k
        off = el * T
        qt, kt = QT[pr], KT[pr]
        st0 = pools["ps"].tile([128, 2, 200], F32, name="st0", tag="ps")
        st1 = pools["ps"].tile([72, 2, 72], F32, name="st1", tag="ps")
        for hl in range(2):
            qh = qt[oc][0:64, hl, off:off + 200]
            kh = kt[oc][0:64, hl, off:off + 200]
            nc.tensor.matmul(st0[:, hl, :], kh[:, 0:128], qh)
            nc.tensor.matmul(st1[:, hl, :], kh[:, 128:200], qh[:, 128:200])
        e_dr = pools["e0"].tile([128, 2, 2, 208], F8, name="e_dr", bufs=3)
        nc.scalar.activation(e_dr[:, 0, :, 0:200], st0[:, :, :], AF.Exp,
                             scale=SCALE)
        nc.scalar.activation(e_dr[0:72, 1, :, 128:200], st1[:, :, :],
                             AF.Exp, scale=SCALE)
        e_sel = pools["e0"].tile([128, 2, 2, 208], F8, name="e_sel", bufs=12)
        nc.gpsimd.memset(e_sel[:, 1, :, :], 0.0)
        nc.gpsimd.affine_select(
            e_sel[:, 0, :, 0:200], e_dr[:, 0, :, 0:200],
            pattern=[[0, 2], [1, 200]], compare_op=AL.is_ge, fill=0.0,
            base=0, channel_multiplier=-1)
        nc.gpsimd.affine_select(
            e_sel[0:72, 1, :, 128:200], e_dr[0:72, 1, :, 128:200],
            pattern=[[0, 2], [1, 72]], compare_op=AL.is_ge, fill=0.0,
            base=0, channel_multiplier=-1)
        ES[(k, oc)] = e_sel

    def emit_av(k, oc):
        v_dr = VV[k]
        es = ES[(k, oc)]
        av = pools["ps"].tile([64, 2, 200], F32, name="av", tag="ps")
        drep = pools["ps"].tile([64, 2, 200], F32, name="drep", tag="ps")
        for hl in range(2):
            h = 2 * oc + hl
            nc.tensor.matmul(av[:, hl, :], v_dr[:, :, h * 64:(h + 1) * 64],
                             es[:, :, hl, 0:200],
                             skip_group_check=True, perf_mode=DR)
        nc.tensor.matmul(drep[:, :, :], onesdr[:, :, :], es[:, :, :, 0:200],
                         skip_group_check=True, perf_mode=DR)
        dinv = pools["dbc"].tile([64, 2, 200], F32, name="dinv", bufs=4)
        nc.vector.reciprocal_approx_fast(dinv[:, :, :], drep[:, :, :])
        ot = pools["ot"].tile([64, 2, 208], F8, name="ot", bufs=16)
        nc.vector.tensor_mul(ot[:, :, 0:200], av[:, :, :], dinv[:, :, :])
        OT[k][oc] = ot

    for i, (k, oc) in enumerate(steps):
        emit_scores(k, oc)
        if i >= LAG:
            emit_av(*steps[i - LAG])
    for i in range(len(steps) - LAG, len(steps)):
        emit_av(*steps[i])

    XN = {}
    for k in keys:
        new_x = []
        for ci, (t0, tc) in enumerate(TCH):
            ps = pools["ps"].tile([tc, E], F32, name="proj_ps", tag="ps")
            for oc in range(4):
                nc.tensor.matmul(ps[:, :], OT[k][oc][0:64, :, t0:t0 + tc],
                                 wo_sb[0:64, 2 * oc:2 * oc + 2, :],
                                 start=(oc == 0), stop=(oc == 3), perf_mode=DR)
            xn = pools["res"].tile([tc, E], F32, name="xn", tag="res")
            nc.vector.scalar_tensor_tensor(xn[:, :], ps[:, :],
                                           1.0 / (WS * WS), XCS[k][ci],
                                           op0=AL.mult, op1=AL.add)
            new_x.append(xn)
        XN[k] = new_x
    return XN


def _build(bpc, stages=3):
    nc = bacc.Bacc("TRN2", target_bir_lowering=False, debug=False,
                   enable_asserts=False, num_devices=NCORES)

    def din(name, shape, dt):
        return nc.dram_tensor(name, list(shape), dt, kind="ExternalInput")

    x_d = din("x", (bpc, 2, 128, E), F32)       # host-padded 200 -> 256 rows
    mem_d = din("mem", (bpc, 2, 128, E), BF16)  # host-padded
    pm_d = din("pm", (bpc // 2, 1, T2), BF16)   # per-pair row
    sm_d = din("sm", (bpc // 2, 1, T2), BF16)   # per-pair row
    wq_sa_d = din("wq_sa", (E, E), F8)
    wk_sa_d = din("wk_sa", (E, E), F8)
    wv_sa_d = din("wv_sa", (E, E), F8)
    wo_sa_d = din("wo_sa", (E, E), F8)
    wq_ca_d = din("wq_ca", (E, E), F8)
    wk_ca_d = din("wk_ca", (E, E), F8)
    wv_ca_d = din("wv_ca", (E, E), F8)
    wo_ca_d = din("wo_ca", (E, E), F8)
    w1_d = din("w1", (E, F), F8)
    w2_d = din("w2", (F, E), F8)
    b1_d = din("b1", (1, F), F32)  # WS*(f_b1 + ln3_b @ f_w1), column bias
    out_d = nc.dram_tensor("out", [bpc, T, E], F32, kind="ExternalOutput")

    identb_d = nc.inline_tensor(np.eye(128, dtype=NPBF16), name="identbc")
    onesdr_np = np.ones((128, 2, 64), dtype=NPF8)
    onesdr_np[72:128, 1, :] = 0  # token-pad rows of key-subtile 1
    onesdr_d = nc.inline_tensor(onesdr_np, name="onesdrc")

    npairs = bpc // 2

    with tile.TileContext(nc) as tcx, ExitStack() as ctx:
        pools = {}

        def pool(name, bufs, space="SBUF"):
            pools[name] = ctx.enter_context(
                tcx.tile_pool(name=name, bufs=bufs, space=space))
            return pools[name]

        wpool = pool("w", 1)
        pool("small", 10)
        pool("h", 8)
        pool("m", 5)
        pool("x2", 6)
        pool("tT", 9)
        pool("qk", 18)
        pool("v", 8)
        pool("e0", 6)
        pool("e1", 6)
        pool("ot", 10)
        pool("dbc", 5)
        pool("res", 15)
        pool("rT", 17)
        pool("mrow", 4)
        pool("ps", 8, space="PSUM")

        eps = wpool.tile([128, 1], F32, tag="eps", bufs=1, name="eps")
        nc.gpsimd.memset(eps[:, :], 1e-5)

        def wtile(name, src, shape, rearr=None, dt=F8, eng=None):
            t = wpool.tile(shape, dt, tag=name, bufs=1, name=name)
            ap = src[:] if rearr is None else src[:].rearrange(rearr, p=shape[0])
            (eng or nc.gpsimd).dma_start(t[...], ap)
            return t

        # consts on sync (tiny, needed early); weights on gpsimd queue
        identb = wtile("identb", identb_d, [128, 128], dt=BF16)
        sel4 = wtile("sel4", sel4_d, [128, 4, 2, 16], dt=F8)
        ones4 = wtile("ones4", ones4_d, [4, 4, 64], dt=BF16)
        wq_sa = wtile("wq_sa", wq_sa_d, [128, 4, E], "(c p) n -> p c n")
        wk_sa = wtile("wk_sa", wk_sa_d, [128, 4, E], "(c p) n -> p c n")
        wv_sa = wtile("wv_sa", wv_sa_d, [128, 4, E], "(c p) n -> p c n")
        wo_sa = wtile("wo_sa", wo_sa_d, [64, 8, E], "(c p) n -> p c n")
        wq_ca = wtile("wq_ca", wq_ca_d, [128, 4, E], "(c p) n -> p c n")
        wk_ca = wtile("wk_ca", wk_ca_d, [128, 4, E], "(c p) n -> p c n")
        wv_ca = wtile("wv_ca", wv_ca_d, [128, 4, E], "(c p) n -> p c n")
        wo_ca = wtile("wo_ca", wo_ca_d, [64, 8, E], "(c p) n -> p c n")
        w1 = wtile("w1", w1_d, [128, 4, F], "(c p) n -> p c n")
        w2 = wtile("w2", w2_d, [128, 16, E], "(c p) n -> p c n")
        b1c = wtile("b1c", b1_d, [128, 16], dt=F32, rearr="o (c p) -> p (o c)")

        GS = {}  # per-group state

        def emit_loads(g):
            st = GS[g] = {"X": {}, "PMBC": {}, "SMBC": {}, "M2": {},
                          "HT": {}, "QT": {}, "KT": {}, "MT": {}}
            for pr in (2 * g, 2 * g + 1):
                els = (2 * pr, 2 * pr + 1)
                x_el = []
                for el, e in enumerate(els):
                    x2 = pools["x2"].tile([128, 2, E], F32, name="x_in",
                                          tag="x2")
                    nc.sync.dma_start(
                        x2[...], x_d[e].rearrange("c p n -> p c n", p=128))
                    x_el.append(x2)
                st["X"][pr] = x_el
                pmrow2 = pools["mrow"].tile([1, T2], BF16, name="pmrow2", bufs=2)
                nc.sync.dma_start(pmrow2[0:1, :], pm_d[pr, :, :])
                pm_bc = pools["mrow"].tile([64, T2], BF16, name="pm_bc")
                nc.gpsimd.partition_broadcast(pm_bc[0:64, :], pmrow2[0:1, :])
                st["PMBC"][pr] = pm_bc
                smrow2 = pools["mrow"].tile([1, T2], BF16, name="smrow2", bufs=2)
                nc.sync.dma_start(smrow2[0:1, :], sm_d[pr, :, :])
                sm_bc = pools["mrow"].tile([64, T2], BF16, name="sm_bc")
                nc.gpsimd.partition_broadcast(sm_bc[0:64, :], smrow2[0:1, :])
                st["SMBC"][pr] = sm_bc

        def emit_sa_ln(g):
            st = GS[g]
            for pr in (2 * g, 2 * g + 1):
                h_pair = _ln_pair(nc, pools, st["X"][pr], eps)
                st["HT"][pr] = _transpose_f8(nc, pools, h_pair, identb)

        def emit_sa_qk(g):
            st = GS[g]
            for pr in (2 * g, 2 * g + 1):
                st["QT"][pr] = _project_qk(nc, pools, wq_sa, st["HT"][pr],
                                           "q_sa", mask_bc=st["PMBC"][pr])
                st["KT"][pr] = _project_qk(nc, pools, wk_sa, st["HT"][pr],
                                           "k_sa", mask_bc=st["PMBC"][pr])

        def emit_sa_attn(g):
            st = GS[g]
            P = (2 * g, 2 * g + 1)
            # mem loads for CA ride along here (gpsimd queue, long lead)
            for pr in P:
                m_el = []
                for el, e in enumerate((2 * pr, 2 * pr + 1)):
                    m2 = pools["m"].tile([128, 2, E], BF16, name="m_nat",
                                         tag="m_nat")
                    nc.gpsimd.dma_start(
                        m2[...], mem_d[e].rearrange("c p n -> p c n", p=128))
                    m_el.append(m2)
                st["M2"][pr] = m_el
            VV = {}
            XCS = {}
            for pr in P:
                for el in range(2):
                    VV[(pr, el)] = _project_v(nc, pools, wv_sa, st["HT"][pr],
                                              el * T, "v_sa")
                    xs = st["X"][pr][el]
                    XCS[(pr, el)] = [xs[0:tc, ci, :]
                                     for ci, (t0, tc) in enumerate(TCH)]
            XN = _attn_stage(nc, pools, P, st["QT"], st["KT"], VV, onesdr,
                             wo_sa, XCS)
            for pr in P:
                st["X"][pr] = [XN[(pr, 0)], XN[(pr, 1)]]

        def emit_ca_prep(g):
            st = GS[g]
            P = (2 * g, 2 * g + 1)
            for pr in P:
                h_pair = _ln_pair(nc, pools, st["X"][pr], eps)
                st["HT"][pr] = _transpose_f8(nc, pools, h_pair, identb)
            for pr in P:
                m_views = [[st["M2"][pr][el][0:128, ci, :] for ci in range(2)]
                           for el in range(2)]
                st["MT"][pr] = _transpose_f8(nc, pools, m_views, identb)
            for pr in P:
                st["QT"][pr] = _project_qk(nc, pools, wq_ca, st["HT"][pr],
                                           "q_ca")
                st["KT"][pr] = _project_qk(nc, pools, wk_ca, st["MT"][pr],
                                           "k_ca", mask_bc=st["SMBC"][pr])

        def emit_ca_attn(g):
            st = GS[g]
            P = (2 * g, 2 * g + 1)
            VV = {}
            XCS = {}
            for pr in P:
                for el in range(2):
                    VV[(pr, el)] = _project_v(nc, pools, wv_ca, st["HT"][pr],
                                              el * T, "v_ca")
                    XCS[(pr, el)] = st["X"][pr][el]
            XN = _attn_stage(nc, pools, P, st["QT"], st["KT"], VV, onesdr,
                             wo_ca, XCS)
            for pr in P:
                st["X"][pr] = [XN[(pr, 0)], XN[(pr, 1)]]

        def emit_ffn(g):
            st = GS[g]
            P = (2 * g, 2 * g + 1)
            for pr in P:
                h_pair = _ln_pair(nc, pools, st["X"][pr], eps)
                st["HT"][pr] = _transpose_f8(nc, pools, h_pair, identb)
            for pr in P:
                rT = []
                for fp in range(8):
                    r = pools["rT"].tile([128, 2, T2], F8, name="r")
                    for sub in range(2):
                        fc = 2 * fp + sub
                        zps = pools["ps"].tile([128, T2], F32, name="z_ps",
                                               tag="ps")
                        nc.tensor.matmul(zps[:, :],
                                         w1[:, 0:2, fc * 128:(fc + 1) * 128],
                                         st["HT"][pr][:, 0:2, :], start=True,
                                         stop=False, perf_mode=DR)
                        nc.tensor.matmul(zps[:, :],
                                         w1[:, 2:4, fc * 128:(fc + 1) * 128],
                                         st["HT"][pr][:, 2:4, :], start=False,
                                         stop=True, perf_mode=DR)
                        nc.scalar.activation(r[:, sub, :], zps[:, :],
                                             AF.Relu, bias=b1c[:, fc:fc + 1])
                    rT.append(r)
                for el in range(2):
                    e = 2 * pr + el
                    off = el * T
                    for ci, (t0, tc) in enumerate(TCH):
                        yps = pools["ps"].tile([tc, E], F32, name="y_ps",
                                               tag="ps")
                        for fp in range(8):
                            nc.tensor.matmul(
                                yps[:, :],
                                rT[fp][:, :, off + t0:off + t0 + tc],
                                w2[:, 2 * fp:2 * fp + 2, :],
                                start=(fp == 0), stop=(fp == 7), perf_mode=DR)
                        yout = pools["res"].tile([tc, E], F32, name="yout",
                                                 tag="res")
                        nc.vector.scalar_tensor_tensor(
                            yout[:, :], yps[:, :], 1.0 / (WS * WS),
                            st["X"][pr][el][ci][:, :], op0=AL.mult, op1=AL.add)
                        nc.gpsimd.dma_start(out_d[e, t0:t0 + tc, :],
                                            yout[:, :])

        # software-pipelined schedule across the two groups
        emit_loads(0)
        emit_sa_ln(0)
        emit_sa_qk(0)
        emit_sa_attn(0)
        emit_loads(1)
        emit_sa_ln(1)
        emit_ca_prep(0)
        emit_ca_attn(0)
        emit_sa_qk(1)
        emit_ffn(0)
        emit_sa_attn(1)
        emit_ca_prep(1)
        emit_ca_attn(1)
        emit_ffn(1)

    nc.compile()
    return nc


def _host_prep(inputs, bpc, core):
    """Build the in_map for one core."""
    s = slice(core * bpc, (core + 1) * bpc)

    def rearr(w, g=None):  # (H, E, D) -> [E, H*D], optionally row-scaled
        m = np.transpose(np.asarray(w, np.float32), (1, 0, 2)).reshape(E, E)
        if g is not None:
            m = m * np.asarray(g, np.float32)[:, None]
        return m

    def f8(a):  # scale x64, clip to TRN fp8e4 range, cast
        return np.clip(np.asarray(a, np.float32) * WS,
                       -240.0, 240.0).astype(NPF8)

    def pad256(a, dt):  # [bpc, T, c] -> [bpc, 2, 128, c]
        out = np.zeros((bpc, 256, a.shape[2]), dtype=dt)
        out[:, :T, :] = a
        return np.ascontiguousarray(out.reshape(bpc, 2, 128, a.shape[2]))

    g1 = np.asarray(inputs["ln1_g"], np.float32)
    g2 = np.asarray(inputs["ln2_g"], np.float32)
    g3 = np.asarray(inputs["ln3_g"], np.float32)
    b3n = np.asarray(inputs["ln3_b"], np.float32)
    w1f = np.asarray(inputs["f_w1"], np.float32)
    b1f = (np.asarray(inputs["f_b1"], np.float32) + b3n @ w1f) * WS

    return {
        "x": pad256(np.asarray(inputs["idx"], np.float32)[s], np.float32),
        "mem": pad256(np.asarray(inputs["memory"], np.float32)[s], NPBF16),
        "pm": np.ascontiguousarray(
            (np.asarray(inputs["pred_mask"])[s] != 0).astype(NPBF16)
            .reshape(bpc // 2, 1, T2)),
        "sm": np.ascontiguousarray(
            (np.asarray(inputs["src_mask"])[s] != 0).astype(NPBF16)
            .reshape(bpc // 2, 1, T2)),
        "wq_sa": f8(rearr(inputs["sa_wq"], g1)),
        "wk_sa": f8(rearr(inputs["sa_wk"], g1)),
        "wv_sa": f8(rearr(inputs["sa_wv"], g1)),
        "wo_sa": f8(inputs["sa_wo"]),
        "wq_ca": f8(rearr(inputs["ca_wq"], g2)),
        "wk_ca": f8(rearr(inputs["ca_wk"])),
        "wv_ca": f8(rearr(inputs["ca_wv"], g2)),
        "wo_ca": f8(inputs["ca_wo"]),
        "w1": f8(w1f * g3[:, None]),
        "w2": f8(inputs["f_w2"]),
        "b1": np.ascontiguousarray(b1f.reshape(1, F)),
    }


def get_program(bpc):
    if bpc not in _programs:
        _programs[bpc] = _build(bpc)
    return _programs[bpc]


def kernel(**inputs) -> np.ndarray:
    bpc = B // NCORES
    nc = get_program(bpc)
    in_maps = [_host_prep(inputs, bpc, c) for c in range(NCORES)]
    res = run_bass_kernel_spmd(nc, in_maps, core_ids=list(range(NCORES)))
    out = np.concatenate([res.results[c]["out"] for c in range(NCORES)], axis=0)
    return out.astype(np.float32)
